# revision 2
# baseline (speedup 1.0000x reference)
"""Distributed linear (ROW_PARALLEL) on 8 TRN2 NeuronCores.

out = (x.fp16 @ weight.fp16.T).fp32 + bias          x:[8192,4096] w:[16384,4096]

Sharding: tensor-parallel over out_features — core i computes the
[8192, 2048] slab out[:, i*2048:(i+1)*2048]; host concatenates.

Per-core device kernel (weight-stationary):
  - w shard [4096, 2048] fp16 = 16 MB resident in SBUF for the whole kernel
  - x streamed per 128-row m-tile, out streamed back per m-tile
  - psum [128, 512] fp32 accumulates 32 k-matmuls (K=4096 = 32 x 128)
  - bias added in fp32 during the psum->sbuf eviction (vector engine)

Host pre-arranges fp16 operands so every DMA is per-partition contiguous.
"""

import numpy as np

import concourse.mybir as mybir
import concourse.tile as tile
from concourse import bacc
from concourse.bass import ts
from concourse.bass_utils import run_bass_kernel_spmd

M, K, N = 8192, 4096, 16384
NCORES = 8
NSH = N // NCORES       # 2048 out-features per core
P = 128
KO = K // P             # 32 k-subtiles
MT = M // P             # 64 m-tiles
NFREE = 512             # psum free dim (one bank, fp32)
NT = NSH // NFREE       # 4 n-tiles per core

_cached = None


def _build():
    nc = bacc.Bacc("TRN2", target_bir_lowering=False, debug=False,
                   num_devices=NCORES)
    xt = nc.dram_tensor("xt", [MT, P, KO, P], mybir.dt.float16,
                        kind="ExternalInput")
    wt = nc.dram_tensor("wt", [P, NT, KO, NFREE], mybir.dt.float16,
                        kind="ExternalInput")
    bb = nc.dram_tensor("bb", [P, NSH], mybir.dt.float32,
                        kind="ExternalInput")
    out = nc.dram_tensor("out", [MT, P, NSH], mybir.dt.float32,
                         kind="ExternalOutput")

    with tile.TileContext(nc) as tc:
        with (
            tc.tile_pool(name="wpool", bufs=1) as wpool,
            tc.tile_pool(name="xpool", bufs=3) as xpool,
            tc.tile_pool(name="opool", bufs=3) as opool,
            tc.tile_pool(name="cpool", bufs=1) as cpool,
            tc.tile_pool(name="pspool", bufs=8, space="PSUM") as pspool,
        ):
            w_sb = wpool.tile([P, NT, KO, NFREE], mybir.dt.float16)
            for nt in range(NT):
                nc.sync.dma_start(w_sb[:, nt], wt[:, nt])
            bias_sb = cpool.tile([P, NSH], mybir.dt.float32)
            nc.sync.dma_start(bias_sb[:], bb[:])

            for mt in range(MT):
                x_sb = xpool.tile([P, KO, P], mybir.dt.float16)
                nc.sync.dma_start(x_sb[:], xt[mt])
                o_sb = opool.tile([P, NSH], mybir.dt.float32)
                for nt in range(NT):
                    ps = pspool.tile([P, NFREE], mybir.dt.float32)
                    for ko in range(KO):
                        nc.tensor.matmul(
                            ps[:], x_sb[:, ko], w_sb[:, nt, ko],
                            start=(ko == 0), stop=(ko == KO - 1),
                        )
                    nc.vector.tensor_add(
                        o_sb[:, ts(nt, NFREE)], ps[:], bias_sb[:, ts(nt, NFREE)]
                    )
                nc.sync.dma_start(out[mt], o_sb[:])

    nc.compile()
    return nc


def _get_nc():
    global _cached
    if _cached is None:
        _cached = _build()
    return _cached


def prep_in_maps(x: np.ndarray, weight: np.ndarray, bias: np.ndarray):
    x16 = np.asarray(x, dtype=np.float16)
    w16 = np.asarray(weight, dtype=np.float16)
    b32 = np.asarray(bias, dtype=np.float32)

    # xt[mt, p, ko, m] = x16[mt*128 + m, ko*128 + p]  (replicated to all cores)
    xt = np.ascontiguousarray(
        x16.reshape(MT, P, KO, P).transpose(0, 3, 2, 1)
    )

    in_maps = []
    for i in range(NCORES):
        wsh = w16[i * NSH:(i + 1) * NSH]              # [2048, 4096]
        # wt[p, nt, ko, nf] = wsh[nt*512 + nf, ko*128 + p]
        wti = np.ascontiguousarray(
            wsh.reshape(NT, NFREE, KO, P).transpose(3, 0, 2, 1)
        )
        bsh = np.ascontiguousarray(
            np.broadcast_to(b32[i * NSH:(i + 1) * NSH], (P, NSH))
        )
        in_maps.append({"xt": xt, "wt": wti, "bb": bsh})
    return in_maps


def kernel(x: np.ndarray, weight: np.ndarray, bias: np.ndarray) -> np.ndarray:
    in_maps = prep_in_maps(x, weight, bias)
    nc = _get_nc()
    res = run_bass_kernel_spmd(nc, in_maps, core_ids=list(range(NCORES)))
    shards = [res.results[i]["out"].reshape(M, NSH) for i in range(NCORES)]
    return np.concatenate(shards, axis=1)


# revision 4
# speedup vs baseline: 2.6345x; 2.6345x over previous
"""Distributed linear (ROW_PARALLEL) on 8 TRN2 NeuronCores.

out = (x.fp16 @ weight.fp16.T).fp32 + bias          x:[8192,4096] w:[16384,4096]

Sharding: tensor-parallel over out_features — core i computes the
[8192, 2048] slab out[:, i*2048:(i+1)*2048]; host concatenates.

Per-core device kernel (weight-stationary, LDW-amortized):
  - w shard [4096, 2048] fp16 = 16 MB resident in SBUF, ko-major layout,
    DMA'd in 8 chunks so compute starts after the first 2 MB
  - x streamed per 128-row m-tile; one LDWEIGHTS (x k-subtile) feeds 4
    matmuls (one per 512-wide n-tile, 4 concurrent psum banks)
  - psum [128, 512] fp32 accumulates 32 k-matmuls (K=4096 = 32 x 128)
  - bias added in fp32 during the psum->sbuf eviction (vector engine)

Host pre-arranges fp16 operands so every DMA is per-partition contiguous.
"""

import numpy as np

import concourse.mybir as mybir
import concourse.tile as tile
from concourse import bacc
from concourse.bass import ts
from concourse.bass_utils import run_bass_kernel_spmd

M, K, N = 8192, 4096, 16384
NCORES = 8
NSH = N // NCORES       # 2048 out-features per core
P = 128
KO = K // P             # 32 k-subtiles
MT = M // P             # 64 m-tiles
NFREE = 512             # psum free dim (one bank, fp32)
NT = NSH // NFREE       # 4 n-tiles per core
W_CHUNK = 4             # ko per w-load DMA chunk

_cached = None


def _build():
    nc = bacc.Bacc("TRN2", target_bir_lowering=False, debug=False,
                   num_devices=NCORES)
    xt = nc.dram_tensor("xt", [MT, P, KO, P], mybir.dt.float16,
                        kind="ExternalInput")
    wt = nc.dram_tensor("wt", [P, KO, NT, NFREE], mybir.dt.float16,
                        kind="ExternalInput")
    bb = nc.dram_tensor("bb", [P, NSH], mybir.dt.float32,
                        kind="ExternalInput")
    out = nc.dram_tensor("out", [MT, P, NSH], mybir.dt.float32,
                         kind="ExternalOutput")

    with tile.TileContext(nc) as tc:
        with (
            tc.tile_pool(name="wpool", bufs=1) as wpool,
            tc.tile_pool(name="xpool", bufs=3) as xpool,
            tc.tile_pool(name="opool", bufs=3) as opool,
            tc.tile_pool(name="cpool", bufs=1) as cpool,
            tc.tile_pool(name="pspool", bufs=2, space="PSUM") as pspool,
        ):
            w_sb = wpool.tile([P, KO, NT, NFREE], mybir.dt.float16)
            for kc in range(0, KO, W_CHUNK):
                nc.sync.dma_start(
                    w_sb[:, kc:kc + W_CHUNK], wt[:, kc:kc + W_CHUNK]
                )
            bias_sb = cpool.tile([P, NSH], mybir.dt.float32)
            nc.sync.dma_start(bias_sb[:], bb[:])

            for mt in range(MT):
                x_sb = xpool.tile([P, KO, P], mybir.dt.float16)
                nc.sync.dma_start(x_sb[:], xt[mt])
                o_sb = opool.tile([P, NSH], mybir.dt.float32)
                # one LDW (x k-subtile) feeds NT matmuls into NT psum banks
                pss = [
                    pspool.tile([P, NFREE], mybir.dt.float32,
                                tag=f"ps{nt}", name=f"ps{nt}")
                    for nt in range(NT)
                ]
                for ko in range(KO):
                    for nt in range(NT):
                        nc.tensor.matmul(
                            pss[nt][:], x_sb[:, ko], w_sb[:, ko, nt],
                            start=(ko == 0), stop=(ko == KO - 1),
                        )
                for nt in range(NT):
                    nc.vector.tensor_add(
                        o_sb[:, ts(nt, NFREE)], pss[nt][:],
                        bias_sb[:, ts(nt, NFREE)],
                    )
                nc.sync.dma_start(out[mt], o_sb[:])

    nc.compile()
    return nc


def _get_nc():
    global _cached
    if _cached is None:
        _cached = _build()
    return _cached


def prep_in_maps(x: np.ndarray, weight: np.ndarray, bias: np.ndarray,
                 dt16=np.float16):
    x16 = np.asarray(x, dtype=dt16)
    w16 = np.asarray(weight, dtype=dt16)
    b32 = np.asarray(bias, dtype=np.float32)

    # xt[mt, p, ko, m] = x16[mt*128 + m, ko*128 + p]  (replicated to all cores)
    xt = np.ascontiguousarray(
        x16.reshape(MT, P, KO, P).transpose(0, 3, 2, 1)
    )

    in_maps = []
    for i in range(NCORES):
        wsh = w16[i * NSH:(i + 1) * NSH]              # [2048, 4096]
        # wt[p, ko, nt, nf] = wsh[nt*512 + nf, ko*128 + p]
        wti = np.ascontiguousarray(
            wsh.reshape(NT, NFREE, KO, P).transpose(3, 2, 0, 1)
        )
        bsh = np.ascontiguousarray(
            np.broadcast_to(b32[i * NSH:(i + 1) * NSH], (P, NSH))
        )
        in_maps.append({"xt": xt, "wt": wti, "bb": bsh})
    return in_maps


def kernel(x: np.ndarray, weight: np.ndarray, bias: np.ndarray) -> np.ndarray:
    in_maps = prep_in_maps(x, weight, bias)
    nc = _get_nc()
    res = run_bass_kernel_spmd(nc, in_maps, core_ids=list(range(NCORES)))
    shards = [res.results[i]["out"].reshape(M, NSH) for i in range(NCORES)]
    return np.concatenate(shards, axis=1)
